# revision 6
# baseline (speedup 1.0000x reference)
"""Behavior-specific PFF (MoE-routed FFN + residual + LayerNorm) on 8 TRN2 cores.

Strategy: expert-parallel with host-side token dispatch.
  - b_seq in [0..4]; 0 = padding (output zeros). Each of the 4 behaviors gets
    2 NeuronCores; its tokens are split between them.
  - Host gathers each core's tokens transposed ([D, C], matmul rhs layout),
    padded to a common capacity C (multiple of 256).
  - Device (per core, same SPMD program, different data):
      hT[f, t]  = relu(W1.T chunks @ xT chunks + b1)        (PE fp32r + ACT)
      psum[t,d] = xT.T (PE transpose, = residual x) [+ 1·b2] + hT.T @ W2.T
      mean,var  = bn_stats/bn_aggr(psum)                    (DVE)
      out       = (psum - mean)/sqrt(var+eps) [*gamma+beta] (DVE)
  - Host scatters per-core outputs back to the full (B, T, D) tensor.

fp32r = Trainium's full-rate fp32 matmul mode (~1e-4 rel err vs fp32).
"""
import math
import numpy as np
from contextlib import ExitStack

import concourse.bacc as bacc
import concourse.tile as tile
import concourse.mybir as mybir
from concourse import bass_utils

F32 = mybir.dt.float32
F32R = mybir.dt.float32r
AF = mybir.ActivationFunctionType
ALU = mybir.AluOpType

D = 512
F = 2048
NB = 4
EPS = 1e-5
NCORES = 8

# test.py introspection hooks (harness never touches these)
LAST_RUN = {}


def _build_nc(C: int, apply_gb: bool, apply_b1: bool, apply_b2: bool,
              mm_dt=F32R):
    """Build + compile the single-core program (run SPMD on 8 cores)."""
    nc = bacc.Bacc("TRN2", target_bir_lowering=False, debug=False,
                   num_devices=NCORES)
    xt_d = nc.dram_tensor("xt", [D, C], mm_dt, kind="ExternalInput").ap()
    w1t_d = nc.dram_tensor("w1t", [D, F], mm_dt, kind="ExternalInput").ap()
    w2t_d = nc.dram_tensor("w2t", [F, D], mm_dt, kind="ExternalInput").ap()
    ident_d = nc.dram_tensor("ident", [128, 128], mm_dt,
                             kind="ExternalInput").ap()
    if apply_b1:
        b1c_d = nc.dram_tensor("b1c", [128, F // 128], F32,
                               kind="ExternalInput").ap()
    if apply_b2:
        b2r_d = nc.dram_tensor("b2r", [1, D], mm_dt,
                               kind="ExternalInput").ap()
        ones_d = nc.dram_tensor("ones", [1, 128], mm_dt,
                                kind="ExternalInput").ap()
    if apply_gb:
        gb_d = nc.dram_tensor("gb", [128, D], F32, kind="ExternalInput").ap()
        bb_d = nc.dram_tensor("bb", [128, D], F32, kind="ExternalInput").ap()
    y_d = nc.dram_tensor("y", [C, D], F32, kind="ExternalOutput").ap()

    KC1 = D // 128    # k-chunks for x @ W1.T (4)
    KC2 = F // 128    # k-chunks for h @ W2.T (16)
    NP = KC2 // 2     # f-tile pairs (8)

    blocks = []
    t0 = 0
    while t0 < C:
        nb = 512 if C - t0 >= 512 else C - t0
        blocks.append((t0, nb))
        t0 += nb

    with tile.TileContext(nc) as tc, ExitStack() as ctx:
        wp = ctx.enter_context(tc.tile_pool(name="wp", bufs=1))
        xtp = ctx.enter_context(tc.tile_pool(name="xtp", bufs=2))
        htp = ctx.enter_context(tc.tile_pool(name="htp", bufs=2))
        rp = ctx.enter_context(tc.tile_pool(name="rp", bufs=3))
        outp = ctx.enter_context(tc.tile_pool(name="outp", bufs=3))
        sp = ctx.enter_context(tc.tile_pool(name="sp", bufs=6))
        ps1 = ctx.enter_context(tc.tile_pool(name="ps1", bufs=3, space="PSUM"))
        ps2 = ctx.enter_context(tc.tile_pool(name="ps2", bufs=2, space="PSUM"))

        def load_xt(t0, nb):
            tiles = []
            for k in range(KC1):
                xt = xtp.tile([128, nb], mm_dt, name=f"xt_{k}", tag=f"xt{k}")
                nc.sync.dma_start(xt[:], xt_d[128 * k:128 * (k + 1), t0:t0 + nb])
                tiles.append(xt)
            return tiles

        # DMA issue order is tuned so the PE never starves on block 0:
        # xt(block0) + the first W1 f-tiles unblock mm1 almost immediately;
        # later W1 chunks and W2 k-chunks stream in under mm1 of block 0.
        xt_next = load_xt(*blocks[0])
        w1_sb = [wp.tile([128, F], mm_dt, name=f"w1_{k}") for k in range(KC1)]
        w2_sb = [wp.tile([128, D], mm_dt, name=f"w2_{k}") for k in range(KC2)]

        for f in (0, 1):
            for k in range(KC1):
                nc.sync.dma_start(
                    w1_sb[k][:, 128 * f:128 * (f + 1)],
                    w1t_d[128 * k:128 * (k + 1), 128 * f:128 * (f + 1)])
        for k in range(KC1):
            nc.sync.dma_start(w1_sb[k][:, 256:512],
                              w1t_d[128 * k:128 * (k + 1), 256:512])

        ident_sb = wp.tile([128, 128], mm_dt, name="ident_sb")
        nc.sync.dma_start(ident_sb[:], ident_d[:])
        eps_sb = wp.tile([128, 1], F32, name="eps_sb")
        nc.vector.memset(eps_sb[:], EPS)
        if apply_b1:
            b1_sb = wp.tile([128, KC2], F32, name="b1_sb")
            nc.sync.dma_start(b1_sb[:], b1c_d[:])
        if apply_b2:
            b2_sb = wp.tile([1, D], mm_dt, name="b2_sb")
            nc.sync.dma_start(b2_sb[:], b2r_d[:])
            ones_sb = wp.tile([1, 128], mm_dt, name="ones_sb")
            nc.sync.dma_start(ones_sb[:], ones_d[:])
        if apply_gb:
            gb_sb = wp.tile([128, D], F32, name="gb_sb")
            nc.sync.dma_start(gb_sb[:], gb_d[:])
            bb_sb = wp.tile([128, D], F32, name="bb_sb")
            nc.sync.dma_start(bb_sb[:], bb_d[:])

        def load_w1_cc(cc):
            for k in range(KC1):
                nc.sync.dma_start(
                    w1_sb[k][:, 512 * cc:512 * (cc + 1)],
                    w1t_d[128 * k:128 * (k + 1), 512 * cc:512 * (cc + 1)])

        load_w1_cc(1)
        for k in range(0, 5):
            nc.sync.dma_start(w2_sb[k][:], w2t_d[128 * k:128 * (k + 1), :])
        load_w1_cc(2)
        for k in range(5, 10):
            nc.sync.dma_start(w2_sb[k][:], w2t_d[128 * k:128 * (k + 1), :])
        load_w1_cc(3)
        for k in range(10, KC2):
            nc.sync.dma_start(w2_sb[k][:], w2t_d[128 * k:128 * (k + 1), :])

        for bi, (t0, nb) in enumerate(blocks):
            xt_t = xt_next
            if bi + 1 < len(blocks):
                xt_next = load_xt(*blocks[bi + 1])

            # mm1: hT[f, t] = relu(W1.T @ x.T + b1), f-tiles processed in
            # pairs sharing one 2-bank PSUM tile and one wide relu.
            ht_t = []
            for j in range(NP):
                p1 = ps1.tile([128, 2, nb], F32, name=f"p1_{j}", tag="p1")
                for s in range(2):
                    f = 2 * j + s
                    for k in range(KC1):
                        nc.tensor.matmul(p1[:, s, :],
                                         w1_sb[k][:, 128 * f:128 * (f + 1)],
                                         xt_t[k][:],
                                         start=(k == 0), stop=(k == KC1 - 1))
                ht = htp.tile([128, 2, nb], mm_dt, name=f"ht_{j}", tag=f"ht{j}")
                if apply_b1:
                    for s in range(2):
                        f = 2 * j + s
                        nc.scalar.activation(ht[:, s, :], p1[:, s, :], AF.Relu,
                                             bias=b1_sb[:, f:f + 1])
                else:
                    nc.scalar.activation(ht[:, :, :], p1[:, :, :], AF.Relu)
                ht_t.append(ht)

            # mm2 + residual-transpose + LN per 128-token tile
            for tt in range(nb // 128):
                sl = slice(128 * tt, 128 * (tt + 1))
                p2 = ps2.tile([128, D], F32, name="p2", tag="p2")
                # residual: psum[t, 128k:128k+128] = xt[k][:, tt].T
                for k in range(KC1):
                    nc.tensor.matmul(
                        p2[:, 128 * k:128 * (k + 1)].bitcast(mm_dt),
                        xt_t[k][:, sl], ident_sb[:],
                        is_transpose=True, start=(k == 0), stop=False,
                        skip_group_check=True)
                if apply_b2:
                    nc.tensor.matmul(p2[:], ones_sb[:], b2_sb[:],
                                     start=False, stop=False,
                                     skip_group_check=True)
                for k in range(KC2):
                    nc.tensor.matmul(p2[:], ht_t[k // 2][:, k % 2, sl],
                                     w2_sb[k][:],
                                     start=False, stop=(k == KC2 - 1),
                                     skip_group_check=True)
                st6 = sp.tile([128, 6], F32, name="st6", tag="st6")
                nc.vector.bn_stats(st6[:], p2[:])
                mv = sp.tile([128, 2], F32, name="mv", tag="mv")
                nc.vector.bn_aggr(mv[:], st6[:])
                stdt = sp.tile([128, 1], F32, name="stdt", tag="stdt")
                nc.scalar.activation(stdt[:], mv[:, 1:2], AF.Sqrt,
                                     bias=eps_sb[:])
                rstd = sp.tile([128, 1], F32, name="rstd", tag="rstd")
                nc.vector.reciprocal(rstd[:], stdt[:])
                nbias = sp.tile([128, 1], F32, name="nbias", tag="nbias")
                nc.vector.scalar_tensor_tensor(nbias[:], mv[:, 0:1], -1.0,
                                               rstd[:], op0=ALU.mult,
                                               op1=ALU.mult)
                o = outp.tile([128, D], F32, name="o", tag="o")
                if apply_gb:
                    t1 = rp.tile([128, D], F32, name="t1", tag="t1")
                    nc.vector.tensor_scalar(t1[:], p2[:], rstd[:], nbias[:],
                                            op0=ALU.mult, op1=ALU.add)
                    t2 = rp.tile([128, D], F32, name="t2", tag="t2")
                    nc.vector.tensor_mul(t2[:], t1[:], gb_sb[:])
                    nc.vector.tensor_add(o[:], t2[:], bb_sb[:])
                else:
                    nc.vector.tensor_scalar(o[:], p2[:], rstd[:], nbias[:],
                                            op0=ALU.mult, op1=ALU.add)
                nc.sync.dma_start(y_d[t0 + 128 * tt:t0 + 128 * (tt + 1), :],
                                  o[:])

    nc.compile()
    return nc


def kernel(x, b_seq, W1, b1, W2, b2, gamma, beta):
    x = np.asarray(x, dtype=np.float32)
    b_seq_np = np.asarray(b_seq)
    W1 = np.asarray(W1, dtype=np.float32)
    b1 = np.asarray(b1, dtype=np.float32)
    W2 = np.asarray(W2, dtype=np.float32)
    b2 = np.asarray(b2, dtype=np.float32)
    gamma = np.asarray(gamma, dtype=np.float32)
    beta = np.asarray(beta, dtype=np.float32)

    B, T, D_ = x.shape
    assert D_ == D and W1.shape == (NB, F, D)
    tokens = np.ascontiguousarray(x.reshape(-1, D))
    bs = b_seq_np.reshape(-1).astype(np.int64)

    # Token dispatch: expert e -> cores 2e and 2e+1.
    idx_per_core = []
    for e in range(NB):
        idx = np.nonzero(bs == e + 1)[0]
        h = (len(idx) + 1) // 2
        idx_per_core.append(idx[:h])
        idx_per_core.append(idx[h:])
    cmax = max(len(i) for i in idx_per_core)
    out = np.zeros_like(tokens)
    if cmax == 0:
        return out.reshape(B, T, D).astype(x.dtype)
    C = max(256, int(math.ceil(cmax / 256.0)) * 256)

    apply_gb = not (np.all(gamma == 1.0) and np.all(beta == 0.0))
    apply_b1 = bool(np.any(b1 != 0.0))
    apply_b2 = bool(np.any(b2 != 0.0))
    nc = _build_nc(C, apply_gb, apply_b1, apply_b2)

    in_maps = []
    for core in range(NCORES):
        e = core // 2
        idx = idx_per_core[core]
        n = len(idx)
        xt = np.zeros((D, C), np.float32)
        xt[:, :n] = tokens[idx].T
        m = {
            "xt": xt,
            "w1t": np.ascontiguousarray(W1[e].T),
            "w2t": np.ascontiguousarray(W2[e].T),
            "ident": np.eye(128, dtype=np.float32),
        }
        if apply_b1:
            m["b1c"] = np.ascontiguousarray(b1[e].reshape(F // 128, 128).T)
        if apply_b2:
            m["b2r"] = b2[e].reshape(1, D).copy()
            m["ones"] = np.ones((1, 128), np.float32)
        if apply_gb:
            m["gb"] = np.ascontiguousarray(
                np.broadcast_to(gamma[e], (128, D)).astype(np.float32))
            m["bb"] = np.ascontiguousarray(
                np.broadcast_to(beta[e], (128, D)).astype(np.float32))
        in_maps.append(m)

    res = bass_utils.run_bass_kernel_spmd(nc, in_maps,
                                          core_ids=list(range(NCORES)))

    for core in range(NCORES):
        idx = idx_per_core[core]
        if len(idx):
            out[idx] = res.results[core]["y"][:len(idx)]

    LAST_RUN["nc"] = nc
    LAST_RUN["in_maps"] = in_maps
    return out.reshape(B, T, D).astype(x.dtype)
